# revision 1
# baseline (speedup 1.0000x reference)
"""Bahdanau-style attention kernel for Trainium2 (8 NeuronCores, batch-parallel).

Computes, for B=16, S=4096, H=512:
    hid  = hidden @ W_attn[:H] + b_attn                       (B, H)
    en   = tanh(hid[:,None,:] + enc @ W_attn[H:])             (B, S, H)
    lg   = en @ v                                             (B, S, 1)
    w    = softmax(lg, axis=1)
    ctx  = w^T @ enc                                          (B, 1, 2H)

Sharding: data-parallel over batch, 2 batches per core. Per core:
  - The big projection enc @ We runs mostly in fp8e4m3 with DoubleRow
    (2 k-subtiles per pass): e-tiles [NF16..7] stream as e4m3 pairs at
    ~1.8x the fp16 rate; the first NF16 e-tiles stay fp16 (read from the
    fp16 copy of enc that the context accumulation needs anyway) to keep
    the quantization noise within the error budget. Both operand sets
    are pre-scaled by 32 so e4m3 stays in its normal range; the tanh
    activation applies scale=1/32 (plus the per-partition hid bias).
  - logits keep the fp16 path: v replicated across 128 partitions as the
    stationary operand so exp(logits) lands pre-broadcast.
  - softmax max-subtraction replaced by a constant shift 8.0 (logits are
    bounded by |v|_1 = 18 since tanh in [-1,1]; exp(logit-8) stays inside
    fp16 normal range), exact after normalization. exp output is fp16 so
    the context weighting reads 16-bit operands.
  - context = sum_s w_s * enc[e, s] via scalar_tensor_tensor fused
    accumulate on the vector engine (1x — the STT opcode has no fast DVE
    mode, so this is the second-busiest engine after the PE).
    ATTN_CTXPE=1 optionally offloads e-tiles 6,7 to the tensor engine via
    an s-major side stream (measured slightly slower: the weight re-layout
    DMAs delay the fp8 stream on the sync queue).
No cross-core communication; output gathered on host.
"""

import os
import numpy as np
import ml_dtypes
from contextlib import ExitStack

import concourse.bacc as bacc
import concourse.tile as tile
from concourse import mybir
from concourse.bass_utils import run_bass_kernel_spmd

F32 = mybir.dt.float32
F16 = mybir.dt.float16
F8 = mybir.dt.float8e4
NP_F8 = ml_dtypes.float8_e4m3

B, S, H = 16, 4096, 512
E = 2 * H                      # 1024 encoder feature dim
NCORES = 8
BPC = B // NCORES              # batches per core = 2
ET = E // 128                  # 8 e-tiles
HT = H // 128                  # 4 h-tiles
SBLK = 512                     # s-block width
NSB = S // SBLK                # 8 s-blocks per batch
KT = H // 128                  # 4 k-tiles for the hidden projection

W_SCALE = 32.0                 # pre-scale on We (both fp8 and fp16 parts)
SHIFT = 8.0                    # softmax logit shift (fp16-safe exp range)

NF16 = int(os.environ.get("ATTN_NF16", "2"))      # leading e-tiles kept fp16
# CTXPE: accumulate the context for the last 2 e-tiles on the tensor engine
# (via an s-major fp16 side stream of those columns + a tiny SBUF->SBUF DMA
# that re-lays the exp weights into s-partition order), offloading 1/4 of
# the vector engine's scalar_tensor_tensor work.
CTXPE = int(os.environ.get("ATTN_CTXPE", "0"))
GROUP_SB = int(os.environ.get("ATTN_GROUP_SB", "2"))

assert NF16 % 2 == 0 and 0 <= NF16 <= 4
NEP8 = (ET - NF16) // 2        # fp8 e-tile pairs
E8 = (ET - NF16) * 128         # fp8 feature columns
NE_STT = ET - 2 if CTXPE else ET   # e-tiles context-accumulated on DVE
EC = 256                           # e-columns handled on the PE (tiles 6,7)
NST = S // 128                     # s-tiles per batch for the PE context

TRACE = False          # set by test harness; harness-default off
LAST_RESULTS = None    # last BassKernelResults (for profiling in test.py)

_NC_CACHE = {}


def _build():
    nc = bacc.Bacc("TRN2", target_bir_lowering=False, debug=False)

    CW = (KT + 1) + KT * BPC            # bshift | hidT, packed (128, CW) f32
    encT = nc.dram_tensor("encT", [BPC, NE_STT * 128, S], F16,
                          kind="ExternalInput").ap()
    enc8 = nc.dram_tensor("enc8", [BPC, E8, S], F8, kind="ExternalInput").ap()
    if CTXPE:
        encS_d = nc.dram_tensor("encS67", [BPC, NST * 128, EC], F16,
                                kind="ExternalInput").ap()
    We16_d = nc.dram_tensor("We16", [128, max(NF16, 1) * H], F16,
                            kind="ExternalInput").ap()
    We8_d = nc.dram_tensor("We8", [128, NEP8 * HT * 2 * 128], F8,
                           kind="ExternalInput").ap()
    V_d = nc.dram_tensor("V128", [128, HT * 128], F16, kind="ExternalInput").ap()
    Wh_d = nc.dram_tensor("Wh16", [128, KT * H], F16, kind="ExternalInput").ap()
    cst_d = nc.dram_tensor("consts", [128, CW], F32, kind="ExternalInput").ap()
    ctx_d = nc.dram_tensor("ctx", [BPC, E], F32, kind="ExternalOutput").ap()

    DR = mybir.MatmulPerfMode.DoubleRow

    with tile.TileContext(nc) as tc, ExitStack() as ctx:
        cpool = ctx.enter_context(tc.tile_pool(name="consts", bufs=1))
        epool = ctx.enter_context(tc.tile_pool(name="enc16", bufs=4))
        e8pool = ctx.enter_context(tc.tile_pool(name="enc8", bufs=4))
        tpool = ctx.enter_context(tc.tile_pool(name="tanh", bufs=2))
        wpool = ctx.enter_context(tc.tile_pool(name="wexp", bufs=2))
        jpool = ctx.enter_context(tc.tile_pool(name="junkv", bufs=2))
        spool = ctx.enter_context(tc.tile_pool(name="stats", bufs=1))
        proj_bufs = 1 if GROUP_SB >= 4 else 2
        pp = ctx.enter_context(tc.tile_pool(name="pproj", bufs=proj_bufs, space="PSUM"))
        pl = ctx.enter_context(tc.tile_pool(name="plog", bufs=2, space="PSUM"))
        ph_pool = ctx.enter_context(tc.tile_pool(name="phid", bufs=1, space="PSUM"))
        if CTXPE:
            s67pool = ctx.enter_context(tc.tile_pool(name="encs67", bufs=3))
            wtpool = ctx.enter_context(tc.tile_pool(name="wt", bufs=3))
            pc_pool = ctx.enter_context(tc.tile_pool(name="pctx", bufs=1,
                                                     space="PSUM"))

        # ---- PE warm-up: dummy matmuls while DMAs land (HAM -> K=8/8) ----
        wlhs = cpool.tile([128, 128], F16)
        wrhs = cpool.tile([128, 256], F16)
        nc.vector.memset(wlhs[:], 0.0)
        nc.vector.memset(wrhs[:], 0.0)
        wps = ph_pool.tile([128, 256], F32, name="warm", tag="ph")
        for _ in range(26):
            nc.tensor.matmul(wps[:], wlhs[:], wrhs[:], start=True, stop=True)

        # ---- constants ----
        cst_sb = cpool.tile([128, CW], F32)
        nc.scalar.dma_start(cst_sb[:], cst_d)
        We8_sb = cpool.tile([128, NEP8 * HT * 2 * 128], F8)
        nc.scalar.dma_start(We8_sb[:], We8_d)
        Wh_sb = cpool.tile([128, KT * H], F16)
        nc.scalar.dma_start(Wh_sb[:], Wh_d)
        We16_sb = cpool.tile([128, max(NF16, 1) * H], F16)
        nc.scalar.dma_start(We16_sb[:], We16_d)
        V_sb = cpool.tile([128, HT * 128], F16)
        nc.scalar.dma_start(V_sb[:], V_d)
        bsh_sb = cst_sb[:, 0:KT + 1]
        hidT16 = cpool.tile([128, KT * BPC], F16)
        nc.gpsimd.tensor_copy(hidT16[:], cst_sb[:, KT + 1:KT + 1 + KT * BPC])

        # ---- hidden projection: hid_sb[:, h*BPC + b] = (hidden @ Wh + b)[b, h-tile]
        hid_sb = spool.tile([128, HT * BPC], F32)
        for h in range(HT):
            ph = ph_pool.tile([128, BPC], F32, name="ph")
            for k in range(KT):
                nc.tensor.matmul(
                    ph[:],
                    Wh_sb[:, k * H + h * 128: k * H + (h + 1) * 128],
                    hidT16[:, k * BPC:(k + 1) * BPC],
                    start=(k == 0), stop=(k == KT - 1),
                )
            nc.vector.tensor_scalar_add(
                hid_sb[:, h * BPC:(h + 1) * BPC], ph[:], bsh_sb[:, h:h + 1])

        # ---- stats accumulators ----
        zslots = spool.tile([128, BPC * NSB], F32)
        cslots = spool.tile([128, BPC * ET * NSB], F32)
        nc.gpsimd.memset(cslots[:], 0.0)

        ctx_red = spool.tile([128, BPC * ET], F32)
        zred = spool.tile([128, BPC], F32)
        zrec = spool.tile([128, BPC], F32)
        ctx_fin = spool.tile([128, BPC * ET], F32)
        if CTXPE:
            pctx = pc_pool.tile([1, BPC * EC], F32)   # PE context accumulator
            c67 = spool.tile([1, BPC * EC], F32)

        groups = []
        pos = 0
        while pos < NSB - 2:
            groups.append(list(range(pos, pos + GROUP_SB)))
            pos += GROUP_SB
        while pos < NSB:
            groups.append([pos])
            pos += 1
        GW = GROUP_SB * SBLK
        T8 = ET - NF16                     # fp8 e-tile count

        ctx_mm_idx = [0, 0]          # per-batch PE-context matmul counter

        def emit_ctx_pe(b, wT, encS, tg):
            """Deferred PE context matmuls for one group (CTXPE)."""
            for t in range(tg):
                k = ctx_mm_idx[b]
                nc.tensor.matmul(
                    pctx[0:1, b * EC:(b + 1) * EC],
                    wT[:, t:t + 1],
                    encS[:, t * EC:(t + 1) * EC],
                    start=(k == 0), stop=(k == NST - 1),
                    skip_group_check=True,
                )
                ctx_mm_idx[b] += 1

        def emit_flush(b, sbs, encg, tanh_t, encS):
            """logits + exp + context accumulation for a finished group."""
            gsb = len(sbs)
            gw = gsb * SBLK
            tg = gw // 128
            wg = wpool.tile([128, GW], F16, name="wg")[:, 0:gw]
            lg = {}
            for i in range(gsb):
                lg[i] = pl.tile([128, SBLK], F32, name=f"logits_{i}", bufs=1)
            for h in range(HT):
                for i in range(gsb):
                    nc.tensor.matmul(
                        lg[i][:], V_sb[:, h * 128:(h + 1) * 128],
                        tanh_t[(h, i)][:],
                        start=(h == 0), stop=(h == HT - 1),
                    )
            for i, sb in enumerate(sbs):
                nc.scalar.activation(
                    wg[:, i * SBLK:(i + 1) * SBLK], lg[i][:],
                    mybir.ActivationFunctionType.Exp,
                    bias=bsh_sb[:, KT:KT + 1],
                    accum_out=zslots[:, b * NSB + sb: b * NSB + sb + 1],
                )
            wT = None
            if CTXPE:
                # re-lay the (replicated) weights into s-partition order:
                # wT[p, t] = w[p*tg + t], matching the host layout of encS.
                # Issued on the gpsimd queue: on sync the ~1.3us descriptor
                # generation delays the fp8 stream (HAM cold-clock stalls),
                # and on scalar it would delay the next group's tanh.
                wT = wtpool.tile([128, GW // 128], F16, name="wt")
                nc.gpsimd.dma_start(wT[:, 0:tg], wg[0:1, 0:gw])
            # context accumulation: one fused op per e-tile. Final groups
            # run per-s-block to keep the tail chain short.
            splits = [(0, gw, sbs[0])] if gsb > 1 else \
                [(i * SBLK, SBLK, sbs[i]) for i in range(gsb)]
            for off, width, sbcol in splits:
                for e in range(NE_STT):
                    src = encg[:, e * gw + off:e * gw + off + width]
                    col = (b * ET + e) * NSB + sbcol
                    jt = jpool.tile([128, GW], F16, name="junkv")
                    nc.vector.scalar_tensor_tensor(
                        jt[:, 0:width], src, 1.0, wg[:, off:off + width],
                        mybir.AluOpType.mult, mybir.AluOpType.mult,
                        accum_out=cslots[:, col:col + 1],
                    )
            return (b, wT, encS, tg)

        def emit_finalize(b):
            """ctx = (sum_sb ctx_partial) / Z for one batch."""
            nc.vector.tensor_reduce(
                ctx_red[:, b * ET:(b + 1) * ET],
                cslots[:, b * ET * NSB:(b + 1) * ET * NSB].rearrange(
                    "p (q s) -> p q s", s=NSB),
                axis=mybir.AxisListType.X, op=mybir.AluOpType.add)
            nc.vector.tensor_reduce(
                zred[:, b:b + 1],
                zslots[:, b * NSB:(b + 1) * NSB].rearrange(
                    "p (q s) -> p q s", s=NSB),
                axis=mybir.AxisListType.X, op=mybir.AluOpType.add)
            nc.vector.reciprocal(zrec[:, b:b + 1], zred[:, b:b + 1])
            nc.vector.tensor_scalar_mul(
                ctx_fin[:, b * ET:(b + 1) * ET],
                ctx_red[:, b * ET:(b + 1) * ET], zrec[:, b:b + 1])
            if CTXPE:
                nc.sync.dma_start(
                    ctx_d[b][0:NE_STT * 128].rearrange("(e p) -> p e", p=128),
                    ctx_fin[:, b * ET:b * ET + NE_STT])
                nc.vector.tensor_copy(
                    c67[0:1, b * EC:(b + 1) * EC],
                    pctx[0:1, b * EC:(b + 1) * EC])
                nc.vector.tensor_scalar_mul(
                    c67[0:1, b * EC:(b + 1) * EC],
                    c67[0:1, b * EC:(b + 1) * EC], zrec[0:1, b:b + 1])
                nc.sync.dma_start(
                    ctx_d[b][NE_STT * 128:E], c67[0:1, b * EC:(b + 1) * EC])
            else:
                nc.sync.dma_start(
                    ctx_d[b].rearrange("(e p) -> p e", p=128),
                    ctx_fin[:, b * ET:(b + 1) * ET])

        ctx_pending = None       # (b, wT, encS, tg, is_batch_last)
        t0s = [0] * BPC
        for b in range(BPC):
            for g, sbs in enumerate(groups):
                t0 = t0s[b]
                gsb = len(sbs)
                gw = gsb * SBLK
                g0 = sbs[0]
                first = (b == 0 and g == 0)
                # fp16 stream: all 8 e-tiles. The blend tiles 0..NF16-1 are
                # MM inputs, so for the very first group they go on the sync
                # queue ahead of everything; otherwise gpsimd queue.
                encg = epool.tile([128, NE_STT * GW], F16, name="encg",
                                  tag="encg")
                if NF16 > 0:
                    chunks16 = [NF16, (NE_STT - NF16 + 1) // 2,
                                (NE_STT - NF16) - (NE_STT - NF16 + 1) // 2]
                else:
                    chunks16 = [NE_STT // 2, NE_STT - NE_STT // 2]
                et0 = 0
                for ci, nt in enumerate(chunks16):
                    eng = nc.sync if (first and ci == 0) else nc.gpsimd
                    eng.dma_start(
                        encg[:, et0 * gw:(et0 + nt) * gw].rearrange(
                            "p (t s) -> p t s", t=nt),
                        encT[b].rearrange("(t p) s -> p t s", p=128)[
                            :, et0:et0 + nt,
                            g0 * SBLK:g0 * SBLK + gw],
                    )
                    et0 += nt
                # fp8 stream: e-tiles NF16..7 (pair-aligned chunks), sync queue
                encg8 = e8pool.tile([128, T8 * GW], F8, name="encg8",
                                    tag="encg8")
                if first:
                    chunks8 = [2, 2] + ([T8 - 4] if T8 > 4 else [])
                else:
                    c0 = (T8 // 2) & ~1 or 2
                    chunks8 = [c0, T8 - c0] if T8 > c0 else [T8]
                et0 = 0
                for nt in chunks8:
                    nc.sync.dma_start(
                        encg8[:, et0 * gw:(et0 + nt) * gw].rearrange(
                            "p (t s) -> p t s", t=nt),
                        enc8[b].rearrange("(t p) s -> p t s", p=128)[
                            :, et0:et0 + nt,
                            g0 * SBLK:g0 * SBLK + gw],
                    )
                    et0 += nt
                encS = None
                if CTXPE:
                    tg = gw // 128
                    encS = s67pool.tile([128, (GW // 128) * EC], F16,
                                        name="encs")
                    nc.gpsimd.dma_start(
                        encS[:, 0:tg * EC].rearrange("p (t e) -> p t e", t=tg),
                        encS_d[b].rearrange("(t p) e -> p t e", p=128)[
                            :, t0:t0 + tg, :])
                    t0s[b] = t0 + tg

                # big projection + tanh, h-tile at a time (fp8 DR first so
                # only the fp8 stream gates the chain start)
                tanh_t = {}
                for h in range(HT):
                    proj = {}
                    for i in range(gsb):
                        proj[i] = pp.tile([128, SBLK], F32, name=f"proj_{i}")
                    for ep in range(NEP8):
                        lhs8 = We8_sb[:, (ep * HT + h) * 256:
                                      (ep * HT + h) * 256 + 256].rearrange(
                            "p (t m) -> p t m", t=2)
                        for i in range(gsb):
                            rhs8 = encg8[:, (2 * ep) * gw:(2 * ep + 2) * gw
                                         ].rearrange("p (t s) -> p t s", t=2)[
                                :, :, i * SBLK:(i + 1) * SBLK]
                            nc.tensor.matmul(
                                proj[i][:], lhs8, rhs8,
                                start=(ep == 0),
                                stop=(ep == NEP8 - 1 and NF16 == 0),
                                perf_mode=DR,
                            )
                    for j in range(NF16):
                        lhs = We16_sb[:, j * H + h * 128: j * H + (h + 1) * 128]
                        for i in range(gsb):
                            nc.tensor.matmul(
                                proj[i][:], lhs,
                                encg[:, j * gw + i * SBLK:
                                     j * gw + (i + 1) * SBLK],
                                start=False, stop=(j == NF16 - 1),
                            )
                    for i in range(gsb):
                        tt = tpool.tile([128, SBLK], F16, name=f"tanh_{h}_{i}")
                        nc.scalar.activation(
                            tt[:], proj[i][:], mybir.ActivationFunctionType.Tanh,
                            bias=hid_sb[:, h * BPC + b: h * BPC + b + 1],
                            scale=1.0 / W_SCALE,
                        )
                        tanh_t[(h, i)] = tt

                # flush this group immediately: the DVE (context STT) is the
                # binding engine, so wg must be produced as early as possible.
                # The PE's short wait for the last tanh tile is affordable.
                # The PE context matmuls are deferred by one group so the PE
                # never waits on the wT re-layout DMA.
                rec = emit_flush(b, sbs, encg, tanh_t, encS)
                if CTXPE:
                    if ctx_pending is not None:
                        pb, pwT, pencS, ptg, plast = ctx_pending
                        emit_ctx_pe(pb, pwT, pencS, ptg)
                        if plast:
                            emit_finalize(pb)
                    ctx_pending = rec + (g == len(groups) - 1,)
                elif g == len(groups) - 1:
                    emit_finalize(b)

        if CTXPE:
            pb, pwT, pencS, ptg, plast = ctx_pending
            emit_ctx_pe(pb, pwT, pencS, ptg)
            emit_finalize(pb)

    nc.compile()
    return nc


def _to_f8(x):
    return np.clip(x, -240.0, 240.0).astype(NP_F8)


def kernel(hidden, encoder_outputs, W_attn, b_attn, v):
    global LAST_RESULTS
    hidden = np.asarray(hidden, dtype=np.float32)
    encoder_outputs = np.asarray(encoder_outputs, dtype=np.float32)
    W_attn = np.asarray(W_attn, dtype=np.float32)
    b_attn = np.asarray(b_attn, dtype=np.float32)
    v = np.asarray(v, dtype=np.float32)

    key = (NF16, CTXPE, GROUP_SB)
    if key not in _NC_CACHE:
        _NC_CACHE[key] = _build()
    nc = _NC_CACHE[key]

    # SBUF-layout packed constants (partition dim = 128 rows)
    We = W_attn[H:] * W_SCALE                        # (E, H), pre-scaled
    We_et = We.reshape(ET, 128, H)                   # e-tile major
    # fp16 part: e-tiles 0..NF16-1, e-major (128, NF16*H)
    if NF16 > 0:
        We16 = np.ascontiguousarray(
            We_et[:NF16].transpose(1, 0, 2).reshape(128, NF16 * H)
        ).astype(np.float16)
    else:
        We16 = np.zeros((128, H), dtype=np.float16)
    # fp8 part: pair-packed (128, NEP8*HT*2*128):
    # block (ep, h) holds [tile 2ep h-cols | tile 2ep+1 h-cols]
    We8_t = We_et[NF16:].reshape(NEP8, 2, 128, HT, 128)
    We8 = _to_f8(np.ascontiguousarray(
        We8_t.transpose(2, 0, 3, 1, 4).reshape(128, NEP8 * HT * 2 * 128)))
    V128 = np.ascontiguousarray(np.broadcast_to(
        v.reshape(HT, 128, 1).transpose(1, 0, 2), (128, HT, 128)
    ).reshape(128, HT * 128).astype(np.float16))
    Wh16 = np.ascontiguousarray(W_attn[:H].reshape(KT, 128, H).transpose(
        1, 0, 2).reshape(128, KT * H).astype(np.float16))
    bsh = np.zeros((128, KT + 1), dtype=np.float32)
    bsh[:, :KT] = b_attn.reshape(KT, 128).T
    bsh[:, KT] = -SHIFT

    # group structure must mirror _build()
    groups = []
    pos = 0
    while pos < NSB - 2:
        groups.append(list(range(pos, pos + GROUP_SB)))
        pos += GROUP_SB
    while pos < NSB:
        groups.append([pos])
        pos += 1

    in_maps = []
    for c in range(NCORES):
        sl = slice(c * BPC, (c + 1) * BPC)
        encTc = np.ascontiguousarray(
            encoder_outputs[sl, :, :NE_STT * 128].transpose(0, 2, 1)
        ).astype(np.float16)
        enc8c = _to_f8(np.ascontiguousarray(
            encoder_outputs[sl, :, NF16 * 128:].transpose(0, 2, 1)))
        hidT_pack = np.ascontiguousarray(
            hidden[sl].T.reshape(KT, 128, BPC).transpose(1, 0, 2)
        ).reshape(128, KT * BPC)
        consts = np.ascontiguousarray(
            np.concatenate([bsh, hidT_pack], axis=1, dtype=np.float32))
        im = {
            "encT": encTc, "enc8": enc8c, "We16": We16, "We8": We8,
            "V128": V128, "consts": consts, "Wh16": Wh16,
        }
        if CTXPE:
            # s-permuted fp16 slab of e-columns 768..1023: for each group,
            # tile t row p holds s = g0*512 + p*tg + t (matches the wT
            # re-layout DMA's linear pairing).
            encS = np.empty((BPC, NST, 128, EC), dtype=np.float16)
            for bb in range(BPC):
                t0 = 0
                for sbs in groups:
                    gw = len(sbs) * SBLK
                    tg = gw // 128
                    slab = encoder_outputs[c * BPC + bb,
                                           sbs[0] * SBLK:sbs[0] * SBLK + gw,
                                           NE_STT * 128:]
                    encS[bb, t0:t0 + tg] = slab.reshape(
                        128, tg, EC).transpose(1, 0, 2)
                    t0 += tg
            im["encS67"] = encS.reshape(BPC, NST * 128, EC)
        in_maps.append(im)

    res = run_bass_kernel_spmd(
        nc, in_maps, core_ids=list(range(NCORES)), trace=TRACE)
    LAST_RESULTS = res

    out = np.empty((B, 1, E), dtype=np.float32)
    for c in range(NCORES):
        out[c * BPC:(c + 1) * BPC, 0, :] = res.results[c]["ctx"]
    return out



# revision 3
# speedup vs baseline: 1.0435x; 1.0435x over previous
"""Bahdanau-style attention kernel for Trainium2 (8 NeuronCores, batch-parallel).

Computes, for B=16, S=4096, H=512:
    hid  = hidden @ W_attn[:H] + b_attn                       (B, H)
    en   = tanh(hid[:,None,:] + enc @ W_attn[H:])             (B, S, H)
    lg   = en @ v                                             (B, S, 1)
    w    = softmax(lg, axis=1)
    ctx  = w^T @ enc                                          (B, 1, 2H)

Sharding: data-parallel over batch, 2 batches per core. Per core the three
hardware resources are balanced at ~80-90us each:
  - PE: the big projection enc @ We runs fully in fp8e4m3 DoubleRow
    (pairs of k-subtiles per pass, operands pre-scaled by 32; the tanh
    applies scale=1/32 + per-partition hid bias). logits keep fp16 with
    v replicated across partitions so exp(logits) lands pre-broadcast.
    e-tiles 6,7 of the context sum also run on the PE: stationary
    softmax-weight columns (re-laid to s-partition order by a tiny
    SBUF->SBUF DMA) against an s-major fp16 side stream.
  - DVE: e-tiles 0..5 of the context accumulate via fused
    scalar_tensor_tensor (1x mode; 1024-wide instructions are optimal).
  - DMA (~25MB/core): fp8 e-major (8 tiles) + fp16 e-major (6 tiles)
    + fp16 s-major (2 tiles), streamed in 1-2 s-block units so the
    first projection matmul starts ~10us in and the tail stays short.
  - softmax max-subtraction replaced by a constant shift 8.0 (logits
    bounded by |v|_1 since tanh in [-1,1]), exact after normalization.
No cross-core communication; output gathered on host.
"""

import os
import numpy as np
import ml_dtypes
from contextlib import ExitStack

import concourse.bacc as bacc
import concourse.tile as tile
from concourse import mybir
from concourse.bass_utils import run_bass_kernel_spmd

F32 = mybir.dt.float32
F16 = mybir.dt.float16
F8 = mybir.dt.float8e4
NP_F8 = ml_dtypes.float8_e4m3

B, S, H = 16, 4096, 512
E = 2 * H                      # 1024 encoder feature dim
NCORES = 8
BPC = B // NCORES              # batches per core = 2
ET = E // 128                  # 8 e-tiles
HT = H // 128                  # 4 h-tiles
SBLK = 512                     # s-block width
NSB = S // SBLK                # 8 s-blocks per batch
KT = H // 128                  # 4 k-tiles for the hidden projection

W_SCALE = 32.0                 # pre-scale on We (both fp8 and fp16 parts)
SHIFT = 8.0                    # softmax logit shift (fp16-safe exp range)

NF16 = int(os.environ.get("ATTN_NF16", "0"))      # leading e-tiles kept fp16
assert NF16 % 2 == 0 and 0 <= NF16 <= 4

NE_DVE = 6                     # e-tiles 0..5 context-accumulated on DVE
NE_PE = ET - NE_DVE            # e-tiles 6,7 context-accumulated on PE
EC = NE_PE * 128               # 256 PE-context feature columns
NST = S // 128                 # 32 s-chunks per batch for the PE context
T8 = ET - NF16                 # fp8 e-tile count
NEP8 = T8 // 2                 # fp8 e-tile pairs (DoubleRow)

# Per-batch unit plan: each unit is 1-2 consecutive s-blocks and is the
# granularity of DMA, STT, weight re-layout and PE-context accumulation.
# Batch 0 starts with two single-block units so the first projection matmul
# only waits on a 512KB fp8 transfer; the tails are single so the last
# DVE chain stays short.
UNITS = [
    [(0,), (1,), (2, 3), (4, 5), (6, 7)],          # batch 0
    [(0, 1), (2, 3), (4, 5), (6,), (7,)],          # batch 1
]
NU = 5                                             # units per batch

TRACE = False          # set by test harness; harness-default off
LAST_RESULTS = None    # last BassKernelResults (for profiling in test.py)

_NC_CACHE = {}


def _build():
    nc = bacc.Bacc("TRN2", target_bir_lowering=False, debug=False)

    CW = (KT + 1) + KT * BPC            # bshift | hidT, packed (128, CW) f32
    encT = nc.dram_tensor("encT", [BPC, NE_DVE * 128, S], F16,
                          kind="ExternalInput").ap()
    enc8 = nc.dram_tensor("enc8", [BPC, T8 * 128, S], F8,
                          kind="ExternalInput").ap()
    encS_d = nc.dram_tensor("encS67", [BPC, NST * 128, EC], F16,
                            kind="ExternalInput").ap()
    We16_d = nc.dram_tensor("We16", [128, max(NF16, 1) * H], F16,
                            kind="ExternalInput").ap()
    We8_d = nc.dram_tensor("We8", [128, NEP8 * HT * 2 * 128], F8,
                           kind="ExternalInput").ap()
    V_d = nc.dram_tensor("V128", [128, HT * 128], F16, kind="ExternalInput").ap()
    Wh_d = nc.dram_tensor("Wh16", [128, KT * H], F16, kind="ExternalInput").ap()
    cst_d = nc.dram_tensor("consts", [128, CW], F32, kind="ExternalInput").ap()
    ctx_d = nc.dram_tensor("ctx", [BPC, E], F32, kind="ExternalOutput").ap()

    DR = mybir.MatmulPerfMode.DoubleRow
    GWMAX = 2 * SBLK

    with tile.TileContext(nc) as tc, ExitStack() as ctx:
        cpool = ctx.enter_context(tc.tile_pool(name="consts", bufs=1))
        epool = ctx.enter_context(tc.tile_pool(name="enc16", bufs=3))
        e8pool = ctx.enter_context(tc.tile_pool(name="enc8", bufs=3))
        s67pool = ctx.enter_context(tc.tile_pool(name="encs67", bufs=3))
        tpool = ctx.enter_context(tc.tile_pool(name="tanh", bufs=2))
        wpool = ctx.enter_context(tc.tile_pool(name="wexp", bufs=2))
        wtpool = ctx.enter_context(tc.tile_pool(name="wt", bufs=3))
        jpool = ctx.enter_context(tc.tile_pool(name="junkv", bufs=2))
        spool = ctx.enter_context(tc.tile_pool(name="stats", bufs=1))
        pp = ctx.enter_context(tc.tile_pool(name="pproj", bufs=2, space="PSUM"))
        pl = ctx.enter_context(tc.tile_pool(name="plog", bufs=2, space="PSUM"))
        ph_pool = ctx.enter_context(tc.tile_pool(name="phid", bufs=1,
                                                 space="PSUM"))
        pc_pool = ctx.enter_context(tc.tile_pool(name="pctx", bufs=1,
                                                 space="PSUM"))

        # ---- constants: We8 first on sync (gates the first matmul); the
        # rest on the scalar queue (idle until the first tanh).
        We8_sb = cpool.tile([128, NEP8 * HT * 2 * 128], F8)
        nc.sync.dma_start(We8_sb[:], We8_d)
        cst_sb = cpool.tile([128, CW], F32)
        nc.scalar.dma_start(cst_sb[:], cst_d)
        Wh_sb = cpool.tile([128, KT * H], F16)
        nc.scalar.dma_start(Wh_sb[:], Wh_d)
        V_sb = cpool.tile([128, HT * 128], F16)
        nc.scalar.dma_start(V_sb[:], V_d)
        We16_sb = cpool.tile([128, max(NF16, 1) * H], F16)
        if NF16 > 0:
            nc.scalar.dma_start(We16_sb[:], We16_d)
        bsh_sb = cst_sb[:, 0:KT + 1]
        hidT16 = cpool.tile([128, KT * BPC], F16)
        nc.gpsimd.tensor_copy(hidT16[:], cst_sb[:, KT + 1:KT + 1 + KT * BPC])

        # ---- PE warm-up: dummy matmuls while DMAs land (HAM -> K=8/8) ----
        wlhs = cpool.tile([128, 128], F16)
        wrhs = cpool.tile([128, 256], F16)
        nc.vector.memset(wlhs[:], 0.0)
        nc.vector.memset(wrhs[:], 0.0)
        wps = pl.tile([128, SBLK], F32, name="lg")
        for _ in range(26):
            nc.tensor.matmul(wps[:, 0:256], wlhs[:], wrhs[:],
                             start=True, stop=True)

        # ---- hidden projection: hid_sb[:, h*BPC + b] = (hidden @ Wh + b)[b, h]
        hid_sb = spool.tile([128, HT * BPC], F32)
        for h in range(HT):
            ph = ph_pool.tile([128, BPC], F32, name="ph")
            for k in range(KT):
                nc.tensor.matmul(
                    ph[:],
                    Wh_sb[:, k * H + h * 128: k * H + (h + 1) * 128],
                    hidT16[:, k * BPC:(k + 1) * BPC],
                    start=(k == 0), stop=(k == KT - 1),
                )
            nc.vector.tensor_scalar_add(
                hid_sb[:, h * BPC:(h + 1) * BPC], ph[:], bsh_sb[:, h:h + 1])

        # ---- stats accumulators (every slot is written; no memset needed)
        zslots = spool.tile([128, BPC * NSB], F32)
        cslots = spool.tile([128, BPC * NE_DVE * NU], F32)
        ctx_red = spool.tile([128, BPC * NE_DVE], F32)
        zred = spool.tile([128, BPC], F32)
        zrec = spool.tile([128, BPC], F32)
        ctx_fin = spool.tile([128, BPC * NE_DVE], F32)
        pctx = pc_pool.tile([1, BPC * EC], F32)    # PE context accumulator
        c67 = spool.tile([1, BPC * EC], F32)

        ctx_mm_idx = [0, 0]          # per-batch PE-context matmul counter

        # flat per-sblock schedule; each sblock knows its unit
        sched = []                   # (b, i, unit_idx, li, unit, t0, new_unit)
        t0s = []
        for b in range(BPC):
            t0 = 0
            for u, unit in enumerate(UNITS[b]):
                for li, i in enumerate(unit):
                    sched.append((b, i, u, li, unit, t0, li == 0))
                t0 += len(unit) * SBLK // 128

        tiles = {}                   # unit -> (encg, enc8g, encS, wT)

        def emit_unit_dma(b, u, unit, t0, first):
            gw = len(unit) * SBLK
            tg = gw // 128
            g0 = unit[0]
            encg = epool.tile([128, NE_DVE * GWMAX], F16, name="encg")
            e8g = e8pool.tile([128, T8 * GWMAX], F8, name="encg8")
            encS = s67pool.tile([128, (GWMAX // 128) * EC], F16, name="encs")
            # fp8 first: it gates the projection chain
            nc.sync.dma_start(
                e8g[:, 0:T8 * gw].rearrange("p (t s) -> p t s", t=T8),
                enc8[b].rearrange("(t p) s -> p t s", p=128)[
                    :, :, g0 * SBLK:g0 * SBLK + gw],
            )
            nc.gpsimd.dma_start(
                encg[:, 0:NE_DVE * gw].rearrange("p (t s) -> p t s", t=NE_DVE),
                encT[b].rearrange("(t p) s -> p t s", p=128)[
                    :, :, g0 * SBLK:g0 * SBLK + gw],
            )
            nc.gpsimd.dma_start(
                encS[:, 0:tg * EC].rearrange("p (t e) -> p t e", t=tg),
                encS_d[b].rearrange("(t p) e -> p t e", p=128)[
                    :, t0:t0 + tg, :])
            return (encg, e8g, encS)

        def emit_proj_tanh(b, u, li, unit):
            gw = len(unit) * SBLK
            encg, e8g, _ = tiles[(b, u)]
            tanh_t = []
            for h in range(HT):
                proj = pp.tile([128, SBLK], F32, name=f"proj_{li % 2}")
                for ep in range(NEP8):
                    lhs8 = We8_sb[:, (ep * HT + h) * 256:
                                  (ep * HT + h) * 256 + 256].rearrange(
                        "p (t m) -> p t m", t=2)
                    rhs8 = e8g[:, (2 * ep) * gw:(2 * ep + 2) * gw].rearrange(
                        "p (t s) -> p t s", t=2)[
                        :, :, li * SBLK:(li + 1) * SBLK]
                    nc.tensor.matmul(
                        proj[:], lhs8, rhs8,
                        start=(ep == 0),
                        stop=(ep == NEP8 - 1 and NF16 == 0),
                        perf_mode=DR,
                    )
                for j in range(NF16):
                    lhs = We16_sb[:, j * H + h * 128: j * H + (h + 1) * 128]
                    nc.tensor.matmul(
                        proj[:], lhs,
                        encg[:, j * gw + li * SBLK: j * gw + (li + 1) * SBLK],
                        start=False, stop=(j == NF16 - 1),
                    )
                tt = tpool.tile([128, SBLK], F16, name=f"tanh_{h}")
                nc.scalar.activation(
                    tt[:], proj[:], mybir.ActivationFunctionType.Tanh,
                    bias=hid_sb[:, h * BPC + b: h * BPC + b + 1],
                    scale=1.0 / W_SCALE,
                )
                tanh_t.append(tt)
            return tanh_t

        def emit_logits_exp(b, i, tanh_t, wg):
            lg = pl.tile([128, SBLK], F32, name="lg")
            for h in range(HT):
                nc.tensor.matmul(
                    lg[:], V_sb[:, h * 128:(h + 1) * 128], tanh_t[h][:],
                    start=(h == 0), stop=(h == HT - 1),
                )
            nc.scalar.activation(
                wg[:, i * SBLK:(i + 1) * SBLK], lg[:],
                mybir.ActivationFunctionType.Exp,
                bias=bsh_sb[:, KT:KT + 1],
                accum_out=zslots[:, b * NSB + i: b * NSB + i + 1],
            )

        def emit_unit_flush(b, u, unit, wg):
            """wT re-layout + DVE context accumulation for a finished unit."""
            gw = len(unit) * SBLK
            tg = gw // 128
            g0 = unit[0]
            encg, _, encS = tiles[(b, u)]
            wT = wtpool.tile([128, GWMAX // 128], F16, name="wt")
            nc.gpsimd.dma_start(
                wT[:, 0:tg], wg[0:1, g0 * SBLK:g0 * SBLK + gw])
            for e in range(NE_DVE):
                jt = jpool.tile([128, GWMAX], F16, name="junkv")
                nc.vector.scalar_tensor_tensor(
                    jt[:, 0:gw], encg[:, e * gw:e * gw + gw], 1.0,
                    wg[:, g0 * SBLK:g0 * SBLK + gw],
                    mybir.AluOpType.mult, mybir.AluOpType.mult,
                    accum_out=cslots[:, (b * NE_DVE + e) * NU + u:
                                     (b * NE_DVE + e) * NU + u + 1],
                )
            return (b, u, unit, wT, encS, tg)

        def emit_ctx_pe(rec):
            """Deferred PE context matmuls for one unit."""
            b, u, unit, wT, encS, tg = rec
            for t in range(tg):
                k = ctx_mm_idx[b]
                nc.tensor.matmul(
                    pctx[0:1, b * EC:(b + 1) * EC],
                    wT[:, t:t + 1],
                    encS[:, t * EC:(t + 1) * EC],
                    start=(k == 0), stop=(k == NST - 1),
                    skip_group_check=True,
                )
                ctx_mm_idx[b] += 1

        def emit_finalize(b):
            """ctx = (sum_u ctx_partial) / Z for one batch."""
            nc.vector.tensor_reduce(
                ctx_red[:, b * NE_DVE:(b + 1) * NE_DVE],
                cslots[:, b * NE_DVE * NU:(b + 1) * NE_DVE * NU].rearrange(
                    "p (q s) -> p q s", s=NU),
                axis=mybir.AxisListType.X, op=mybir.AluOpType.add)
            nc.vector.tensor_reduce(
                zred[:, b:b + 1],
                zslots[:, b * NSB:(b + 1) * NSB].rearrange(
                    "p (q s) -> p q s", s=NSB),
                axis=mybir.AxisListType.X, op=mybir.AluOpType.add)
            nc.vector.reciprocal(zrec[:, b:b + 1], zred[:, b:b + 1])
            nc.vector.tensor_scalar_mul(
                ctx_fin[:, b * NE_DVE:(b + 1) * NE_DVE],
                ctx_red[:, b * NE_DVE:(b + 1) * NE_DVE], zrec[:, b:b + 1])
            nc.sync.dma_start(
                ctx_d[b][0:NE_DVE * 128].rearrange("(e p) -> p e", p=128),
                ctx_fin[:, b * NE_DVE:(b + 1) * NE_DVE])
            nc.vector.tensor_copy(
                c67[0:1, b * EC:(b + 1) * EC],
                pctx[0:1, b * EC:(b + 1) * EC])
            nc.vector.tensor_scalar_mul(
                c67[0:1, b * EC:(b + 1) * EC],
                c67[0:1, b * EC:(b + 1) * EC], zrec[0:1, b:b + 1])
            nc.sync.dma_start(
                ctx_d[b][NE_DVE * 128:E], c67[0:1, b * EC:(b + 1) * EC])

        # ---- main software-pipelined loop over all 16 s-blocks ----
        wgs = [None, None]
        pend = None          # (b, i, u, li, unit, tanh_t) awaiting logits/exp
        ctx_pending = None   # unit record awaiting PE-context matmuls
        fin_pending = None   # batch awaiting finalize after its last ctx MMs
        for (b, i, u, li, unit, t0, new_unit) in sched:
            if new_unit:
                tiles[(b, u)] = emit_unit_dma(b, u, unit, t0, u == 0)
            if wgs[b] is None:
                wgs[b] = wpool.tile([128, S], F16, name="wg")
            tanh_t = emit_proj_tanh(b, u, li, unit)
            if pend is not None:
                pb, pi, pu, pli, punit, ptanh = pend
                emit_logits_exp(pb, pi, ptanh, wgs[pb])
                if pli == len(punit) - 1:
                    rec = emit_unit_flush(pb, pu, punit, wgs[pb])
                    if ctx_pending is not None:
                        emit_ctx_pe(ctx_pending)
                        if fin_pending is not None:
                            emit_finalize(fin_pending)
                            fin_pending = None
                    ctx_pending = rec
                    if pu == NU - 1:
                        fin_pending = pb
            pend = (b, i, u, li, unit, tanh_t)
        # drain the pipeline
        pb, pi, pu, pli, punit, ptanh = pend
        emit_logits_exp(pb, pi, ptanh, wgs[pb])
        rec = emit_unit_flush(pb, pu, punit, wgs[pb])
        emit_ctx_pe(ctx_pending)
        if fin_pending is not None:
            emit_finalize(fin_pending)
        emit_ctx_pe(rec)
        emit_finalize(pb)

    nc.compile()
    return nc


def _to_f8(x):
    return np.clip(x, -240.0, 240.0).astype(NP_F8)


def kernel(hidden, encoder_outputs, W_attn, b_attn, v):
    global LAST_RESULTS
    hidden = np.asarray(hidden, dtype=np.float32)
    encoder_outputs = np.asarray(encoder_outputs, dtype=np.float32)
    W_attn = np.asarray(W_attn, dtype=np.float32)
    b_attn = np.asarray(b_attn, dtype=np.float32)
    v = np.asarray(v, dtype=np.float32)

    key = (NF16,)
    if key not in _NC_CACHE:
        _NC_CACHE[key] = _build()
    nc = _NC_CACHE[key]

    # SBUF-layout packed constants (partition dim = 128 rows)
    We = W_attn[H:] * W_SCALE                        # (E, H), pre-scaled
    We_et = We.reshape(ET, 128, H)                   # e-tile major
    if NF16 > 0:
        We16 = np.ascontiguousarray(
            We_et[:NF16].transpose(1, 0, 2).reshape(128, NF16 * H)
        ).astype(np.float16)
    else:
        We16 = np.zeros((128, H), dtype=np.float16)
    # fp8 part: pair-packed (128, NEP8*HT*2*128):
    # block (ep, h) holds [tile 2ep h-cols | tile 2ep+1 h-cols]
    We8_t = We_et[NF16:].reshape(NEP8, 2, 128, HT, 128)
    We8 = _to_f8(np.ascontiguousarray(
        We8_t.transpose(2, 0, 3, 1, 4).reshape(128, NEP8 * HT * 2 * 128)))
    V128 = np.ascontiguousarray(np.broadcast_to(
        v.reshape(HT, 128, 1).transpose(1, 0, 2), (128, HT, 128)
    ).reshape(128, HT * 128).astype(np.float16))
    Wh16 = np.ascontiguousarray(W_attn[:H].reshape(KT, 128, H).transpose(
        1, 0, 2).reshape(128, KT * H).astype(np.float16))
    bsh = np.zeros((128, KT + 1), dtype=np.float32)
    bsh[:, :KT] = b_attn.reshape(KT, 128).T
    bsh[:, KT] = -SHIFT

    in_maps = []
    for c in range(NCORES):
        sl = slice(c * BPC, (c + 1) * BPC)
        encTc = np.ascontiguousarray(
            encoder_outputs[sl, :, :NE_DVE * 128].transpose(0, 2, 1)
        ).astype(np.float16)
        enc8c = _to_f8(np.ascontiguousarray(
            encoder_outputs[sl, :, NF16 * 128:].transpose(0, 2, 1)))
        hidT_pack = np.ascontiguousarray(
            hidden[sl].T.reshape(KT, 128, BPC).transpose(1, 0, 2)
        ).reshape(128, KT * BPC)
        consts = np.ascontiguousarray(
            np.concatenate([bsh, hidT_pack], axis=1, dtype=np.float32))
        # s-permuted fp16 slab of e-columns 768..1023: for each unit,
        # chunk t row p holds s = s0 + p*tg + t (matches the wT
        # re-layout DMA's linear pairing).
        encS = np.empty((BPC, NST, 128, EC), dtype=np.float16)
        for bb in range(BPC):
            t0 = 0
            for unit in UNITS[bb]:
                gw = len(unit) * SBLK
                tg = gw // 128
                s0 = unit[0] * SBLK
                slab = encoder_outputs[c * BPC + bb, s0:s0 + gw,
                                       NE_DVE * 128:]
                encS[bb, t0:t0 + tg] = slab.reshape(
                    128, tg, EC).transpose(1, 0, 2)
                t0 += tg
        im = {
            "encT": encTc, "enc8": enc8c, "We16": We16, "We8": We8,
            "V128": V128, "consts": consts, "Wh16": Wh16,
            "encS67": encS.reshape(BPC, NST * 128, EC),
        }
        in_maps.append(im)

    res = run_bass_kernel_spmd(
        nc, in_maps, core_ids=list(range(NCORES)), trace=TRACE)
    LAST_RESULTS = res

    out = np.empty((B, 1, E), dtype=np.float32)
    for c in range(NCORES):
        out[c * BPC:(c + 1) * BPC, 0, :] = res.results[c]["ctx"]
    return out


# revision 18
# speedup vs baseline: 1.0766x; 1.0317x over previous
"""Bahdanau-style attention kernel for Trainium2 (8 NeuronCores, batch-parallel).

Computes, for B=16, S=4096, H=512:
    hid  = hidden @ W_attn[:H] + b_attn                       (B, H)
    en   = tanh(hid[:,None,:] + enc @ W_attn[H:])             (B, S, H)
    lg   = en @ v                                             (B, S, 1)
    w    = softmax(lg, axis=1)
    ctx  = w^T @ enc                                          (B, 1, 2H)

Sharding: data-parallel over batch, 2 batches per core. Per core the three
hardware resources are balanced at ~80-90us each:
  - PE: the big projection enc @ We runs fully in fp8e4m3 DoubleRow
    (pairs of k-subtiles per pass, operands pre-scaled by 32; the tanh
    applies scale=1/32 + per-partition hid bias). logits keep fp16 with
    v replicated across partitions so exp(logits) lands pre-broadcast.
    e-tiles 6,7 of the context sum also run on the PE: stationary
    softmax-weight columns (re-laid to s-partition order by a tiny
    SBUF->SBUF DMA) against an s-major fp16 side stream.
  - DVE: e-tiles 0..5 of the context accumulate via fused
    scalar_tensor_tensor (1x mode; 1024-wide instructions are optimal).
  - DMA (~25MB/core): fp8 e-major (8 tiles) + fp16 e-major (6 tiles)
    + fp16 s-major (2 tiles), streamed in 1-2 s-block units so the
    first projection matmul starts ~10us in and the tail stays short.
  - softmax max-subtraction replaced by a constant shift 8.0 (logits
    bounded by |v|_1 since tanh in [-1,1]), exact after normalization.
No cross-core communication; output gathered on host.
"""

import os
import numpy as np
import ml_dtypes
from contextlib import ExitStack

import concourse.bacc as bacc
import concourse.tile as tile
from concourse import mybir
from concourse.bass_utils import run_bass_kernel_spmd

F32 = mybir.dt.float32
F16 = mybir.dt.float16
F8 = mybir.dt.float8e4
NP_F8 = ml_dtypes.float8_e4m3

B, S, H = 16, 4096, 512
E = 2 * H                      # 1024 encoder feature dim
NCORES = 8
BPC = B // NCORES              # batches per core = 2
ET = E // 128                  # 8 e-tiles
HT = H // 128                  # 4 h-tiles
SBLK = 512                     # s-block width
NSB = S // SBLK                # 8 s-blocks per batch
KT = H // 128                  # 4 k-tiles for the hidden projection

W_SCALE = 32.0                 # pre-scale on We (both fp8 and fp16 parts)
SHIFT = 8.0                    # softmax logit shift (fp16-safe exp range)

NF16 = int(os.environ.get("ATTN_NF16", "0"))      # leading e-tiles kept fp16
assert NF16 % 2 == 0 and 0 <= NF16 <= 4

NE_DVE = 6                     # e-tiles 0..5 context-accumulated on DVE
NE_PE = ET - NE_DVE            # e-tiles 6,7 context-accumulated on PE
EC = NE_PE * 128               # 256 PE-context feature columns
NST = S // 128                 # 32 s-chunks per batch for the PE context
T8 = ET - NF16                 # fp8 e-tile count
NEP8 = T8 // 2                 # fp8 e-tile pairs (DoubleRow)

# Per-batch unit plan: each unit is 1-2 consecutive s-blocks and is the
# granularity of DMA, STT, weight re-layout and PE-context accumulation.
# Batch 0 starts with two single-block units so the first projection matmul
# only waits on a 512KB fp8 transfer; the tails are single so the last
# DVE chain stays short.
UNITS = [
    [(0,), (1,), (2, 3), (4, 5), (6, 7)],          # batch 0
    [(0,), (1, 2), (3, 4), (5, 6), (7,)],          # batch 1
]
NU = 5                                             # units per batch

TRACE = False          # set by test harness; harness-default off
LAST_RESULTS = None    # last BassKernelResults (for profiling in test.py)

_NC_CACHE = {}


def _build():
    nc = bacc.Bacc("TRN2", target_bir_lowering=False, debug=False)

    CW = (KT + 1) + KT * BPC            # bshift | hidT, packed (128, CW) f32
    encT = nc.dram_tensor("encT", [BPC, NE_DVE * 128, S], F16,
                          kind="ExternalInput").ap()
    enc8 = nc.dram_tensor("enc8", [BPC, T8 * 128, S], F8,
                          kind="ExternalInput").ap()
    encS_d = nc.dram_tensor("encS67", [BPC, NST * 128, EC], F16,
                            kind="ExternalInput").ap()
    We16_d = nc.dram_tensor("We16", [128, max(NF16, 1) * H], F16,
                            kind="ExternalInput").ap()
    We8_d = nc.dram_tensor("We8", [128, NEP8 * HT * 2 * 128], F8,
                           kind="ExternalInput").ap()
    V_d = nc.dram_tensor("V128", [128, HT * 128], F16, kind="ExternalInput").ap()
    Wh_d = nc.dram_tensor("Wh16", [128, KT * H], F16, kind="ExternalInput").ap()
    cst_d = nc.dram_tensor("consts", [128, CW], F32, kind="ExternalInput").ap()
    ctx_d = nc.dram_tensor("ctx", [BPC, E], F32, kind="ExternalOutput").ap()

    DR = mybir.MatmulPerfMode.DoubleRow
    GWMAX = 2 * SBLK

    with tile.TileContext(nc) as tc, ExitStack() as ctx:
        cpool = ctx.enter_context(tc.tile_pool(name="consts", bufs=1))
        epool = ctx.enter_context(tc.tile_pool(name="enc16", bufs=4))
        e8pool = ctx.enter_context(tc.tile_pool(name="enc8", bufs=4))
        s67pool = ctx.enter_context(tc.tile_pool(name="encs67", bufs=4))
        tpool = ctx.enter_context(tc.tile_pool(name="tanh", bufs=2))
        wpool = ctx.enter_context(tc.tile_pool(name="wexp", bufs=2))
        wtpool = ctx.enter_context(tc.tile_pool(name="wt", bufs=3))
        jpool = ctx.enter_context(tc.tile_pool(name="junkv", bufs=2))
        spool = ctx.enter_context(tc.tile_pool(name="stats", bufs=1))
        pp = ctx.enter_context(tc.tile_pool(name="pproj", bufs=2, space="PSUM"))
        pl = ctx.enter_context(tc.tile_pool(name="plog", bufs=2, space="PSUM"))
        ph_pool = ctx.enter_context(tc.tile_pool(name="phid", bufs=1,
                                                 space="PSUM"))
        pc_pool = ctx.enter_context(tc.tile_pool(name="pctx", bufs=1,
                                                 space="PSUM"))

        # ---- constants: We8 first on sync (gates the first matmul); the
        # rest on the scalar queue (idle until the first tanh).
        We8_sb = cpool.tile([128, NEP8 * HT * 2 * 128], F8)
        nc.sync.dma_start(We8_sb[:], We8_d)
        cst_sb = cpool.tile([128, CW], F32)
        nc.scalar.dma_start(cst_sb[:], cst_d)
        Wh_sb = cpool.tile([128, KT * H], F16)
        nc.scalar.dma_start(Wh_sb[:], Wh_d)
        V_sb = cpool.tile([128, HT * 128], F16)
        nc.scalar.dma_start(V_sb[:], V_d)
        We16_sb = cpool.tile([128, max(NF16, 1) * H], F16)
        if NF16 > 0:
            nc.scalar.dma_start(We16_sb[:], We16_d)
        bsh_sb = cst_sb[:, 0:KT + 1]
        hidT16 = cpool.tile([128, KT * BPC], F16)
        nc.gpsimd.tensor_copy(hidT16[:], cst_sb[:, KT + 1:KT + 1 + KT * BPC])

        # ---- PE warm-up: dummy matmuls while DMAs land (HAM -> K=8/8) ----
        wlhs = cpool.tile([128, 128], F16)
        wrhs = cpool.tile([128, 256], F16)
        nc.vector.memset(wlhs[:], 0.0)
        nc.vector.memset(wrhs[:], 0.0)
        wps = pl.tile([128, SBLK], F32, name="lg")
        for _ in range(20):
            nc.tensor.matmul(wps[:, 0:256], wlhs[:], wrhs[:],
                             start=True, stop=True)

        # ---- hidden projection: hid_sb[:, h*BPC + b] = (hidden @ Wh + b)[b, h]
        hid_sb = spool.tile([128, HT * BPC], F32)
        for h in range(HT):
            ph = ph_pool.tile([128, BPC], F32, name="ph")
            for k in range(KT):
                nc.tensor.matmul(
                    ph[:],
                    Wh_sb[:, k * H + h * 128: k * H + (h + 1) * 128],
                    hidT16[:, k * BPC:(k + 1) * BPC],
                    start=(k == 0), stop=(k == KT - 1),
                )
            nc.vector.tensor_scalar_add(
                hid_sb[:, h * BPC:(h + 1) * BPC], ph[:], bsh_sb[:, h:h + 1])

        # ---- stats accumulators (every slot is written; no memset needed)
        zslots = spool.tile([128, BPC * NSB], F32)
        cslots = spool.tile([128, BPC * NE_DVE * NU], F32)
        ctx_red = spool.tile([128, BPC * NE_DVE], F32)
        zred = spool.tile([128, BPC], F32)
        zrec = spool.tile([128, BPC], F32)
        ctx_fin = spool.tile([128, BPC * NE_DVE], F32)
        pctx = pc_pool.tile([1, BPC * EC], F32)    # PE context accumulator
        c67 = spool.tile([1, BPC * EC], F32)

        ctx_mm_idx = [0, 0]          # per-batch PE-context matmul counter

        # flat per-sblock schedule; each sblock knows its unit
        sched = []                   # (b, i, unit_idx, li, unit, t0, new_unit)
        t0s = []
        for b in range(BPC):
            t0 = 0
            for u, unit in enumerate(UNITS[b]):
                for li, i in enumerate(unit):
                    sched.append((b, i, u, li, unit, t0, li == 0))
                t0 += len(unit) * SBLK // 128

        tiles = {}                   # unit -> (encg, enc8g, encS, wT)

        def emit_dma8(b, unit, e8g, q8):
            gw = len(unit) * SBLK
            g0 = unit[0]
            et0 = 0
            for nt in [2] * (T8 // 2):
                q8.dma_start(
                    e8g[:, et0 * gw:(et0 + nt) * gw].rearrange(
                        "p (t s) -> p t s", t=nt),
                    enc8[b].rearrange("(t p) s -> p t s", p=128)[
                        :, et0:et0 + nt, g0 * SBLK:g0 * SBLK + gw],
                )
                et0 += nt

        def emit_dma16(b, unit, t0, encg, encS):
            gw = len(unit) * SBLK
            tg = gw // 128
            g0 = unit[0]
            chunks16 = [1] * NE_DVE if gw > SBLK else [2, 2, 2]
            et0 = 0
            for nt in chunks16:
                nc.gpsimd.dma_start(
                    encg[:, et0 * gw:(et0 + nt) * gw].rearrange(
                        "p (t s) -> p t s", t=nt),
                    encT[b].rearrange("(t p) s -> p t s", p=128)[
                        :, et0:et0 + nt, g0 * SBLK:g0 * SBLK + gw],
                )
                et0 += nt
            nc.sync.dma_start(
                encS[:, 0:tg * EC].rearrange("p (t e) -> p t e", t=tg),
                encS_d[b].rearrange("(t p) e -> p t e", p=128)[
                    :, t0:t0 + tg, :])

        def emit_unit_dma(b, u, unit, t0, first):
            # Each dma_start's transfer rides a single DMA ring (~29 GB/s);
            # aggregate bandwidth comes from many in-flight instructions.
            # Chunk every stream to ~256KB. fp8 gates the projection chain,
            # so it always goes first; the first two units' fp8 generate on
            # sync and gpsimd in parallel so the pipeline fills fast.
            encg = epool.tile([128, NE_DVE * GWMAX], F16, name="encg")
            e8g = e8pool.tile([128, T8 * GWMAX], F8, name="encg8")
            encS = s67pool.tile([128, (GWMAX // 128) * EC], F16, name="encs")
            emit_dma8(b, unit, e8g, nc.sync)
            emit_dma16(b, unit, t0, encg, encS)
            return (encg, e8g, encS, unit, t0)

        def emit_proj_tanh(b, u, li, unit):
            gw = len(unit) * SBLK
            encg, e8g = tiles[(b, u)][0], tiles[(b, u)][1]
            tanh_t = []
            for h in range(HT):
                proj = pp.tile([128, SBLK], F32, name=f"proj_{li % 2}")
                for ep in range(NEP8):
                    lhs8 = We8_sb[:, (ep * HT + h) * 256:
                                  (ep * HT + h) * 256 + 256].rearrange(
                        "p (t m) -> p t m", t=2)
                    rhs8 = e8g[:, (2 * ep) * gw:(2 * ep + 2) * gw].rearrange(
                        "p (t s) -> p t s", t=2)[
                        :, :, li * SBLK:(li + 1) * SBLK]
                    nc.tensor.matmul(
                        proj[:], lhs8, rhs8,
                        start=(ep == 0),
                        stop=(ep == NEP8 - 1 and NF16 == 0),
                        perf_mode=DR,
                    )
                for j in range(NF16):
                    lhs = We16_sb[:, j * H + h * 128: j * H + (h + 1) * 128]
                    nc.tensor.matmul(
                        proj[:], lhs,
                        encg[:, j * gw + li * SBLK: j * gw + (li + 1) * SBLK],
                        start=False, stop=(j == NF16 - 1),
                    )
                tt = tpool.tile([128, SBLK], F16, name=f"tanh_{h}")
                nc.scalar.activation(
                    tt[:], proj[:], mybir.ActivationFunctionType.Tanh,
                    bias=hid_sb[:, h * BPC + b: h * BPC + b + 1],
                    scale=1.0 / W_SCALE,
                )
                tanh_t.append(tt)
            return tanh_t

        def emit_logits_exp(b, i, tanh_t, wg):
            lg = pl.tile([128, SBLK], F32, name="lg")
            for h in range(HT):
                nc.tensor.matmul(
                    lg[:], V_sb[:, h * 128:(h + 1) * 128], tanh_t[h][:],
                    start=(h == 0), stop=(h == HT - 1),
                )
            nc.scalar.activation(
                wg[:, i * SBLK:(i + 1) * SBLK], lg[:],
                mybir.ActivationFunctionType.Exp,
                bias=bsh_sb[:, KT:KT + 1],
                accum_out=zslots[:, b * NSB + i: b * NSB + i + 1],
            )

        def emit_unit_flush(b, u, unit, wg):
            """wT re-layout + DVE context accumulation for a finished unit."""
            gw = len(unit) * SBLK
            tg = gw // 128
            g0 = unit[0]
            encg, encS = tiles[(b, u)][0], tiles[(b, u)][2]
            wT = wtpool.tile([128, GWMAX // 128], F16, name="wt")
            nc.gpsimd.dma_start(
                wT[:, 0:tg], wg[0:1, g0 * SBLK:g0 * SBLK + gw])
            for e in range(NE_DVE):
                jt = jpool.tile([128, GWMAX], F16, name="junkv")
                nc.vector.scalar_tensor_tensor(
                    jt[:, 0:gw], encg[:, e * gw:e * gw + gw], 1.0,
                    wg[:, g0 * SBLK:g0 * SBLK + gw],
                    mybir.AluOpType.mult, mybir.AluOpType.mult,
                    accum_out=cslots[:, (b * NE_DVE + e) * NU + u:
                                     (b * NE_DVE + e) * NU + u + 1],
                )
            return (b, u, unit, wT, encS, tg)

        def emit_ctx_pe(rec):
            """Deferred PE context matmuls for one unit."""
            b, u, unit, wT, encS, tg = rec
            for t in range(tg):
                k = ctx_mm_idx[b]
                nc.tensor.matmul(
                    pctx[0:1, b * EC:(b + 1) * EC],
                    wT[:, t:t + 1],
                    encS[:, t * EC:(t + 1) * EC],
                    start=(k == 0), stop=(k == NST - 1),
                    skip_group_check=True,
                )
                ctx_mm_idx[b] += 1

        def emit_finalize(b):
            """ctx = (sum_u ctx_partial) / Z for one batch."""
            nc.vector.tensor_reduce(
                ctx_red[:, b * NE_DVE:(b + 1) * NE_DVE],
                cslots[:, b * NE_DVE * NU:(b + 1) * NE_DVE * NU].rearrange(
                    "p (q s) -> p q s", s=NU),
                axis=mybir.AxisListType.X, op=mybir.AluOpType.add)
            nc.vector.tensor_reduce(
                zred[:, b:b + 1],
                zslots[:, b * NSB:(b + 1) * NSB].rearrange(
                    "p (q s) -> p q s", s=NSB),
                axis=mybir.AxisListType.X, op=mybir.AluOpType.add)
            nc.vector.reciprocal(zrec[:, b:b + 1], zred[:, b:b + 1])
            nc.vector.tensor_scalar_mul(
                ctx_fin[:, b * NE_DVE:(b + 1) * NE_DVE],
                ctx_red[:, b * NE_DVE:(b + 1) * NE_DVE], zrec[:, b:b + 1])
            nc.sync.dma_start(
                ctx_d[b][0:NE_DVE * 128].rearrange("(e p) -> p e", p=128),
                ctx_fin[:, b * NE_DVE:(b + 1) * NE_DVE])
            nc.vector.tensor_copy(
                c67[0:1, b * EC:(b + 1) * EC],
                pctx[0:1, b * EC:(b + 1) * EC])
            nc.vector.tensor_scalar_mul(
                c67[0:1, b * EC:(b + 1) * EC],
                c67[0:1, b * EC:(b + 1) * EC], zrec[0:1, b:b + 1])
            nc.sync.dma_start(
                ctx_d[b][NE_DVE * 128:E], c67[0:1, b * EC:(b + 1) * EC])

        # ---- main software-pipelined loop over all 16 s-blocks ----
        wgs = [None, None]
        pend = None          # (b, i, u, li, unit, tanh_t) awaiting logits/exp
        ctx_pending = None   # unit record awaiting PE-context matmuls
        fin_pending = None   # batch awaiting finalize after its last ctx MMs
        for (b, i, u, li, unit, t0, new_unit) in sched:
            if new_unit:
                tiles[(b, u)] = emit_unit_dma(b, u, unit, t0, u == 0)
            if wgs[b] is None:
                wgs[b] = wpool.tile([128, S], F16, name="wg")
            tanh_t = emit_proj_tanh(b, u, li, unit)
            if pend is not None:
                pb, pi, pu, pli, punit, ptanh = pend
                emit_logits_exp(pb, pi, ptanh, wgs[pb])
                if pli == len(punit) - 1:
                    rec = emit_unit_flush(pb, pu, punit, wgs[pb])
                    if ctx_pending is not None:
                        emit_ctx_pe(ctx_pending)
                        if fin_pending is not None:
                            emit_finalize(fin_pending)
                            fin_pending = None
                    ctx_pending = rec
                    if pu == NU - 1:
                        fin_pending = pb
            pend = (b, i, u, li, unit, tanh_t)
        # drain the pipeline
        pb, pi, pu, pli, punit, ptanh = pend
        emit_logits_exp(pb, pi, ptanh, wgs[pb])
        rec = emit_unit_flush(pb, pu, punit, wgs[pb])
        emit_ctx_pe(ctx_pending)
        if fin_pending is not None:
            emit_finalize(fin_pending)
        emit_ctx_pe(rec)
        emit_finalize(pb)

    nc.compile()
    return nc


def _to_f8(x):
    return np.clip(x, -240.0, 240.0).astype(NP_F8)


def kernel(hidden, encoder_outputs, W_attn, b_attn, v):
    global LAST_RESULTS
    hidden = np.asarray(hidden, dtype=np.float32)
    encoder_outputs = np.asarray(encoder_outputs, dtype=np.float32)
    W_attn = np.asarray(W_attn, dtype=np.float32)
    b_attn = np.asarray(b_attn, dtype=np.float32)
    v = np.asarray(v, dtype=np.float32)

    key = (NF16,)
    if key not in _NC_CACHE:
        _NC_CACHE[key] = _build()
    nc = _NC_CACHE[key]

    # SBUF-layout packed constants (partition dim = 128 rows)
    We = W_attn[H:] * W_SCALE                        # (E, H), pre-scaled
    We_et = We.reshape(ET, 128, H)                   # e-tile major
    if NF16 > 0:
        We16 = np.ascontiguousarray(
            We_et[:NF16].transpose(1, 0, 2).reshape(128, NF16 * H)
        ).astype(np.float16)
    else:
        We16 = np.zeros((128, H), dtype=np.float16)
    # fp8 part: pair-packed (128, NEP8*HT*2*128):
    # block (ep, h) holds [tile 2ep h-cols | tile 2ep+1 h-cols]
    We8_t = We_et[NF16:].reshape(NEP8, 2, 128, HT, 128)
    We8 = _to_f8(np.ascontiguousarray(
        We8_t.transpose(2, 0, 3, 1, 4).reshape(128, NEP8 * HT * 2 * 128)))
    V128 = np.ascontiguousarray(np.broadcast_to(
        v.reshape(HT, 128, 1).transpose(1, 0, 2), (128, HT, 128)
    ).reshape(128, HT * 128).astype(np.float16))
    Wh16 = np.ascontiguousarray(W_attn[:H].reshape(KT, 128, H).transpose(
        1, 0, 2).reshape(128, KT * H).astype(np.float16))
    bsh = np.zeros((128, KT + 1), dtype=np.float32)
    bsh[:, :KT] = b_attn.reshape(KT, 128).T
    bsh[:, KT] = -SHIFT

    in_maps = []
    for c in range(NCORES):
        sl = slice(c * BPC, (c + 1) * BPC)
        encTc = np.ascontiguousarray(
            encoder_outputs[sl, :, :NE_DVE * 128].transpose(0, 2, 1)
        ).astype(np.float16)
        enc8c = _to_f8(np.ascontiguousarray(
            encoder_outputs[sl, :, NF16 * 128:].transpose(0, 2, 1)))
        hidT_pack = np.ascontiguousarray(
            hidden[sl].T.reshape(KT, 128, BPC).transpose(1, 0, 2)
        ).reshape(128, KT * BPC)
        consts = np.ascontiguousarray(
            np.concatenate([bsh, hidT_pack], axis=1, dtype=np.float32))
        # s-permuted fp16 slab of e-columns 768..1023: for each unit,
        # chunk t row p holds s = s0 + p*tg + t (matches the wT
        # re-layout DMA's linear pairing).
        encS = np.empty((BPC, NST, 128, EC), dtype=np.float16)
        for bb in range(BPC):
            t0 = 0
            for unit in UNITS[bb]:
                gw = len(unit) * SBLK
                tg = gw // 128
                s0 = unit[0] * SBLK
                slab = encoder_outputs[c * BPC + bb, s0:s0 + gw,
                                       NE_DVE * 128:]
                encS[bb, t0:t0 + tg] = slab.reshape(
                    128, tg, EC).transpose(1, 0, 2)
                t0 += tg
        im = {
            "encT": encTc, "enc8": enc8c, "We16": We16, "We8": We8,
            "V128": V128, "consts": consts, "Wh16": Wh16,
            "encS67": encS.reshape(BPC, NST * 128, EC),
        }
        in_maps.append(im)

    res = run_bass_kernel_spmd(
        nc, in_maps, core_ids=list(range(NCORES)), trace=TRACE)
    LAST_RESULTS = res

    out = np.empty((B, 1, E), dtype=np.float32)
    for c in range(NCORES):
        out[c * BPC:(c + 1) * BPC, 0, :] = res.results[c]["ctx"]
    return out
